# revision 38
# baseline (speedup 1.0000x reference)
"""Multi-head self-attention (B=4, N=2048, C=1024, H=16) on 8 trn2 cores.

Sharding: core c -> (batch b = c // 2, head-group g = c % 2).
Each core: Q/K/V projections for its 512 channels, softmax attention for its
8 heads, partial output projection through its 512 rows of Wo; host sums the
two partials per batch (plus bo).

Design (v12, 441us vs the 874us v3 baseline):
  - fp16 x / weights / q / k / v / probs / ctxT; fp32 PSUM accumulation.
  - scores: two concurrent row-split matmuls (head A in PE rows 0:63, head B
    in rows 64:127, one 512-cycle pass for both) -> one [128, 2, 512] PSUM
    tile per key-tile; one [128, 1024] exp per key-tile covering both heads.
  - exp split across engines: 11/16 key-tiles on ScalarE (table exp writing
    fp16), 5/16 on the DVE as an integer Schraudolph exp (bits =
    round(SCH_A*score + SCH_B) written as uint16 = the fp16 bit pattern).
  - softmax row-sums come FREE from a 65th all-ones column appended to V
    (65-wide ctx matmul writes ctx rows 0:63 + the sum in row 64).
  - normalize: stage the sum row to partition 0 (reciprocal_approx_fast
    ignores the input partition offset), fast reciprocal, gpsimd partition
    broadcast, DVE multiplies (partition-shifted write for head B).
  - ctx PSUM is a 3-slot ring of SEPARATE tiles (tile-granular dependency
    tracking would otherwise serialize each qc behind the previous
    normalize); head B's accumulation lags 3 key-tiles so the slot handoff
    is off the critical path; scores+exp are software-pipelined one tile
    ahead across qc/pair boundaries.
  - the freed 8th PSUM bank hosts fillers that keep the PE busy (and the
    HAM clock-gate at 8/8): V projection streaming just ahead of pair 0,
    Q/K projections for pair p+1 during pair p, and the output projection
    for pair 3's previous q-chunk during pair 3.
  - PE warm-up: dummy matmuls during the initial x DMA flip the HAM
    clock-gate to 8/8 before the first projection.
"""

import numpy as np

B, N, C, H = 4, 2048, 1024, 16
D = C // H            # 64
G = 2                 # head-groups (tensor-parallel factor)
J = C // G            # 512 local channels
HL = H // G           # 8 local heads
CT = C // 128         # 8 c-tiles
JT = J // 128         # 4 local j-tiles (head pairs)
NT = N // 128         # 16 token tiles
KT = N // 128         # 16 key tiles
QC = 512              # q-chunk width
NQC = N // QC         # 4 q-chunks
HC = 512              # projection chunk width
NHC = N // HC         # 4 chunks
N_CORES = 8

# key-tiles whose exp runs on the DVE as an integer Schraudolph exp writing
# fp16 bit patterns (offloads ~1/3 of the exp work from the ScalarE).
# Pairs 0 and 3 carry extra DVE load (V-projection writes / output-projection
# copies), so they offload fewer tiles.
SCH_BY_PAIR = {0: (5, 11), 1: (2, 5, 8, 11, 14), 2: (2, 5, 8, 11, 14),
               3: (5, 11)}
SCH_A = float(1024 * 0.125 * np.log2(np.e))   # per raw-score unit
SCH_B = 15301.0                               # calibrated for fp16 bits

_CACHE = {}


def _build():
    import sys
    if "/opt/trn_rl_repo" not in sys.path:
        sys.path.insert(0, "/opt/trn_rl_repo")
    from contextlib import ExitStack
    import concourse.bacc as bacc
    import concourse.tile as tile
    from concourse import mybir

    f32 = mybir.dt.float32
    f16 = mybir.dt.float16
    u16 = mybir.dt.uint16
    Exp = mybir.ActivationFunctionType.Exp
    mult = mybir.AluOpType.mult
    add = mybir.AluOpType.add

    nc = bacc.Bacc("TRN2", target_bir_lowering=False, debug=False)

    xT_d = nc.dram_tensor("xT", [C, N], f16, kind="ExternalInput")
    wq_d = nc.dram_tensor("wq", [C, J], f16, kind="ExternalInput")
    wk_d = nc.dram_tensor("wk", [C, J], f16, kind="ExternalInput")
    wv_d = nc.dram_tensor("wv", [C, J], f16, kind="ExternalInput")
    wo_d = nc.dram_tensor("wo", [J, C], f16, kind="ExternalInput")
    bq_d = nc.dram_tensor("bq", [J], f32, kind="ExternalInput")
    bk_d = nc.dram_tensor("bk", [J], f32, kind="ExternalInput")
    bv_d = nc.dram_tensor("bv", [J], f32, kind="ExternalInput")
    y_d = nc.dram_tensor("y", [N, C], f32, kind="ExternalOutput")

    xT_r = xT_d.ap().rearrange("(ct p) n -> p ct n", p=128)

    with tile.TileContext(nc) as tc, ExitStack() as top:
        consts = top.enter_context(tc.tile_pool(name="consts", bufs=1))
        persist = top.enter_context(tc.tile_pool(name="persist", bufs=1))
        etp = top.enter_context(tc.tile_pool(name="etp", bufs=8))
        ysbp = top.enter_context(tc.tile_pool(name="ysb", bufs=3))
        rr1p = top.enter_context(tc.tile_pool(name="rr1p", bufs=3))
        rrbp = top.enter_context(tc.tile_pool(name="rrbp", bufs=3))

        # ---- persistent SBUF ----
        xt_t = persist.tile([128, CT, N], f16, tag="xt")
        qt_t = persist.tile([128, JT, N], f16, tag="qt")
        kt_t = persist.tile([128, JT, N], f16, tag="kt")
        v_t = persist.tile([128, KT, HL, 66], f16, tag="v")
        ctxT_t = persist.tile([128, JT, N], f16, tag="ctxT")
        wq_t = persist.tile([128, CT, J], f16, tag="wq")
        wk_t = persist.tile([128, CT, J], f16, tag="wk")
        wv_t = persist.tile([128, CT, J], f16, tag="wv")
        wo_t = persist.tile([128, JT, C], f16, tag="wo")

        bq_t = consts.tile([128, JT], f32, tag="bq")
        bk_t = consts.tile([128, JT], f32, tag="bk")
        bv_t = consts.tile([128, J], f32, tag="bv")
        warm_t = consts.tile([128, 8], f32, tag="warm")
        warm_o = consts.tile([128, 8], f16, tag="warmo")

        # ---- DMAs (x on the sync queue, weights on the gpsimd queue) ----
        for ct in range(CT):
            nc.sync.dma_start(out=xt_t[:, ct, :], in_=xT_r[:, ct, :])
        for ct in range(CT):
            nc.gpsimd.dma_start(out=wq_t[:, ct, :], in_=wq_d.ap()[ct * 128:(ct + 1) * 128, :])
            nc.gpsimd.dma_start(out=wk_t[:, ct, :], in_=wk_d.ap()[ct * 128:(ct + 1) * 128, :])
            nc.gpsimd.dma_start(out=wv_t[:, ct, :], in_=wv_d.ap()[ct * 128:(ct + 1) * 128, :])
        for jt in range(JT):
            nc.gpsimd.dma_start(out=wo_t[:, jt, :], in_=wo_d.ap()[jt * 128:(jt + 1) * 128, :])
        nc.sync.dma_start(out=bq_t[:], in_=bq_d.ap().rearrange("(t p) -> p t", p=128))
        nc.sync.dma_start(out=bk_t[:], in_=bk_d.ap().rearrange("(t p) -> p t", p=128))
        nc.sync.dma_start(
            out=bv_t[:], in_=bv_d.ap().unsqueeze(0).partition_broadcast(128).squeeze(1)
        )

        nc.vector.memset(warm_t[:], 0.0)
        nc.scalar.activation(warm_o[:], warm_t[:], Exp)  # load exp table early
        nc.vector.memset(v_t[:, :, :, 64:65], 1.0)       # ones col -> sum in row 64

        def qk_chunk_gen(jt, pool, h, on_scalar=False):
            ns = h * HC
            Ident = mybir.ActivationFunctionType.Identity
            for w_t, b_t, o_t in ((wq_t, bq_t, qt_t), (wk_t, bk_t, kt_t)):
                ps = pool.tile([128, HC], f32, tag="qk")
                for ct in range(CT):
                    nc.tensor.matmul(
                        ps[:], w_t[:, ct, jt * 128:(jt + 1) * 128],
                        xt_t[:, ct, ns:ns + HC],
                        start=(ct == 0), stop=(ct == CT - 1),
                    )
                    yield
                if on_scalar:
                    # keep the PSUM-release off the DVE queue (it would sit
                    # behind the normalize chain and stall the PE fillers)
                    nc.scalar.activation(
                        o_t[:, jt, ns:ns + HC], ps[:], Ident,
                        bias=b_t[:, jt:jt + 1],
                    )
                else:
                    nc.vector.tensor_scalar_add(
                        o_t[:, jt, ns:ns + HC], ps[:], b_t[:, jt:jt + 1]
                    )

        def yproj_gen(qc, pool):
            for nt in range(4 * qc, 4 * qc + 4):
                for cc in range(2):
                    y_ps = pool.tile([128, 512], f32, tag="qk")
                    for jt in range(JT):
                        nc.tensor.matmul(
                            y_ps[:],
                            ctxT_t[:, jt, nt * 128:(nt + 1) * 128],
                            wo_t[:, jt, cc * 512:(cc + 1) * 512],
                            start=(jt == 0), stop=(jt == JT - 1),
                        )
                        yield
                    y_sb = ysbp.tile([128, 512], f32, tag="ysb")
                    nc.vector.tensor_copy(y_sb[:], y_ps[:])
                    # alternate store queues: halves the final DMA drain
                    (nc.sync if cc == 0 else nc.gpsimd).dma_start(
                        out=y_d.ap()[nt * 128:(nt + 1) * 128,
                                     cc * 512:(cc + 1) * 512],
                        in_=y_sb[:],
                    )

        def drain(gen, n=10 ** 9):
            for _ in range(n):
                if next(gen, "END") == "END":
                    return True
            return False

        def qk_pass(jt, pool):
            for h in range(NHC):
                drain(qk_chunk_gen(jt, pool, h))

        # ---- pass 0: Q/K for j-tile 0 (with PE warm-up during the x DMA:
        #      ~4us of dummy matmuls flips the HAM clock gate to 8/8) ----
        warm16 = consts.tile([128, 512], f16, tag="warm16")
        nc.vector.memset(warm16[:], 0.0)
        with tc.tile_pool(name="p0ps", bufs=2, space="PSUM") as p0ps:
            wu_ps = p0ps.tile([128, 512], f32, tag="qk")
            for r in range(24):
                nc.tensor.matmul(
                    wu_ps[:], warm16[:, 0:128], warm16[:],
                    start=(r == 0), stop=(r == 23),
                )
            qk_pass(0, p0ps)
            for nt in range(4):
                v_ps = p0ps.tile([128, J], f32, tag="qk")
                for ct in range(CT):
                    nc.tensor.matmul(
                        v_ps[:], xt_t[:, ct, nt * 128:(nt + 1) * 128],
                        wv_t[:, ct, :], start=(ct == 0), stop=(ct == CT - 1),
                    )
                nc.vector.tensor_tensor(
                    v_t[:, nt, :, 0:64],
                    v_ps[:].rearrange("p (h d) -> p h d", h=HL),
                    bv_t[:].rearrange("p (h d) -> p h d", h=HL),
                    add,
                )

        # ---- attention (pair p) interleaved with Q/K projections (p+1);
        #      the V projection streams through the qki bank just ahead of
        #      its first consumers in (pair 0, qc 0) ----
        with (
            tc.tile_pool(name="stp", bufs=2, space="PSUM") as stp,
            tc.tile_pool(name="cxps", bufs=1, space="PSUM") as cxps,
            tc.tile_pool(name="qki", bufs=1, space="PSUM") as qki,
        ):
            for nt in range(4, NT):
                v_ps = qki.tile([128, J], f32, tag="qk")
                for ct in range(CT):
                    nc.tensor.matmul(
                        v_ps[:], xt_t[:, ct, nt * 128:(nt + 1) * 128],
                        wv_t[:, ct, :], start=(ct == 0), stop=(ct == CT - 1),
                    )
                nc.vector.tensor_tensor(
                    v_t[:, nt, :, 0:64],
                    v_ps[:].rearrange("p (h d) -> p h d", h=HL),
                    bv_t[:].rearrange("p (h d) -> p h d", h=HL),
                    add,
                )
            # 3-slot ring as separate tiles (separate dependency domains —
            # a single tile would serialize each qc's ctx behind the
            # previous qc's normalize reads)
            cx_slots = []
            for s in range(3):
                cx_slot = cxps.tile([128, QC], f32, tag=f"cx{s}", name=f"cx{s}")
                cx_slots.append(cx_slot)

            def emit_scores_exp(p, qc, k):
                qs = qc * QC
                st_ps = stp.tile([128, 2, QC], f32, tag="st")
                nc.tensor.matmul(
                    st_ps[:, 0, :],
                    kt_t[0:64, p, k * 128:(k + 1) * 128],
                    qt_t[0:64, p, qs:qs + QC],
                    start=True, stop=True,
                )
                nc.tensor.matmul(
                    st_ps[:, 1, :],
                    kt_t[64:128, p, k * 128:(k + 1) * 128],
                    qt_t[64:128, p, qs:qs + QC],
                    start=True, stop=True,
                )
                et_t = etp.tile([128, 2, QC], f16, tag="et")
                if k in SCH_BY_PAIR[p]:
                    nc.vector.tensor_scalar(
                        et_t[:].bitcast(u16), st_ps[:], SCH_A, SCH_B, mult, add
                    )
                else:
                    nc.scalar.activation(et_t[:], st_ps[:], Exp, scale=0.125)
                return et_t

            units = [(p, qc) for p in range(JT) for qc in range(NQC)]
            et_next = emit_scores_exp(0, 0, 0)
            for ui, (p, qc) in enumerate(units):
                hA, hB = 2 * p, 2 * p + 1
                i = ui
                sA, sB = (2 * i) % 3, (2 * i + 1) % 3
                qs = qc * QC
                if p + 1 < JT:
                    filler = qk_chunk_gen(p + 1, qki, qc)
                    per_k = 3
                elif qc > 0:
                    filler = yproj_gen(qc - 1, qki)
                    per_k = 3
                else:
                    filler = iter(())
                    per_k = 0
                pend_b = []
                for k in range(KT):
                    et_t = et_next
                    # software pipeline: launch the NEXT tile's scores+exp
                    # (crossing qc/pair boundaries) before this tile's ctx
                    if k + 1 < KT:
                        et_next = emit_scores_exp(p, qc, k + 1)
                    elif ui + 1 < len(units):
                        p2, qc2 = units[ui + 1]
                        et_next = emit_scores_exp(p2, qc2, 0)
                    first, last = (k == 0), (k == KT - 1)
                    with tc.high_priority():
                        nc.tensor.matmul(
                            cx_slots[sA][0:65, :], v_t[:, k, hA, 0:65],
                            et_t[:, 0, :], start=first, stop=last,
                        )
                    # lag head B by 3 k-tiles: its k=0 matmul reuses the
                    # ring slot the previous qc's normalize just released
                    pend_b.append((k, et_t))
                    if len(pend_b) > 3:
                        kb, et_b = pend_b.pop(0)
                        with tc.high_priority():
                            nc.tensor.matmul(
                                cx_slots[sB][0:65, :], v_t[:, kb, hB, 0:65],
                                et_b[:, 1, :], start=(kb == 0),
                                stop=(kb == KT - 1),
                            )
                    drain(filler, per_k)
                for kb, et_b in pend_b:
                    with tc.high_priority():
                        nc.tensor.matmul(
                            cx_slots[sB][0:65, :], v_t[:, kb, hB, 0:65],
                            et_b[:, 1, :], start=(kb == 0),
                            stop=(kb == KT - 1),
                        )
                drain(filler)
                # normalize: slot A first (it is reused soonest).
                # reciprocal_approx_fast only reads partition 0 correctly,
                # so stage the sum row there first.
                for s, h0, pr in ((sA, 0, hA), (sB, 1, hB)):
                    rr1_t = rr1p.tile([1, QC], f32, tag="rr1")
                    sc_t = rr1p.tile([1, QC], f32, tag="sc")
                    nc.vector.tensor_copy(sc_t[0:1, :], cx_slots[s][64:65, :])
                    nc.vector.reciprocal_approx_fast(
                        rr1_t[0:1, :], sc_t[0:1, :]
                    )
                    rrb_t = rrbp.tile([128, QC], f32, tag="rrb")
                    nc.gpsimd.partition_broadcast(rrb_t[:], rr1_t[0:1, :])
                    po = 64 * h0
                    nc.vector.tensor_tensor(
                        ctxT_t[po:po + 64, p, qs:qs + QC],
                        cx_slots[s][0:64, :],
                        rrb_t[po:po + 64, :], mult,
                    )
        with tc.tile_pool(name="yfl", bufs=4, space="PSUM") as yfl:
            # dependency-free bridge matmuls: the flush must wait ~4us for the
            # last normalize, which is one full HAM window — without these the
            # clock gate re-throttles and the flush runs at 1.2 GHz
            br_ps = yfl.tile([128, 512], f32, tag="qk")
            for r in range(14):
                nc.tensor.matmul(
                    br_ps[:], warm16[:, 0:128], warm16[:],
                    start=(r == 0), stop=(r == 13),
                )
            with tc.high_priority():
                drain(yproj_gen(NQC - 1, yfl))

    nc.compile()
    return nc


def _get_module():
    if "nc" not in _CACHE:
        _CACHE["nc"] = _build()
    return _CACHE["nc"]


def _in_maps(x, Wq, bq, Wk, bk, Wv, bv, Wo):
    f16 = np.float16
    maps = []
    for c in range(N_CORES):
        b, g = divmod(c, 2)
        js = slice(g * J, (g + 1) * J)
        maps.append({
            "xT": np.ascontiguousarray(x[b].T.astype(f16)),
            "wq": np.ascontiguousarray(Wq[:, js].astype(f16)),
            "wk": np.ascontiguousarray(Wk[:, js].astype(f16)),
            "wv": np.ascontiguousarray(Wv[:, js].astype(f16)),
            "wo": np.ascontiguousarray(Wo[js, :].astype(f16)),
            "bq": np.ascontiguousarray(bq[js].astype(np.float32)),
            "bk": np.ascontiguousarray(bk[js].astype(np.float32)),
            "bv": np.ascontiguousarray(bv[js].astype(np.float32)),
        })
    return maps


def kernel(x, Wq, bq, Wk, bk, Wv, bv, Wo, bo, **_unused):
    import sys
    if "/opt/trn_rl_repo" not in sys.path:
        sys.path.insert(0, "/opt/trn_rl_repo")
    from concourse.bass_utils import run_bass_kernel_spmd

    x = np.asarray(x, dtype=np.float32)
    Wq = np.asarray(Wq, dtype=np.float32)
    Wk = np.asarray(Wk, dtype=np.float32)
    Wv = np.asarray(Wv, dtype=np.float32)
    Wo = np.asarray(Wo, dtype=np.float32)
    bq = np.asarray(bq, dtype=np.float32)
    bk = np.asarray(bk, dtype=np.float32)
    bv = np.asarray(bv, dtype=np.float32)
    bo = np.asarray(bo, dtype=np.float32)

    nc = _get_module()
    res = run_bass_kernel_spmd(
        nc, _in_maps(x, Wq, bq, Wk, bk, Wv, bv, Wo), list(range(N_CORES))
    )
    out = np.empty((B, N, C), dtype=np.float32)
    for b in range(B):
        out[b] = res.results[2 * b]["y"] + res.results[2 * b + 1]["y"] + bo
    return out


# revision 39
# speedup vs baseline: 1.0101x; 1.0101x over previous
"""Multi-head self-attention (B=4, N=2048, C=1024, H=16) on 8 trn2 cores.

Sharding: core c -> (batch b = c // 2, head-group g = c % 2).
Each core: Q/K/V projections for its 512 channels, softmax attention for its
8 heads, partial output projection through its 512 rows of Wo; host sums the
two partials per batch (plus bo).

Design (v12, 441us vs the 874us v3 baseline):
  - fp16 x / weights / q / k / v / probs / ctxT; fp32 PSUM accumulation.
  - scores: two concurrent row-split matmuls (head A in PE rows 0:63, head B
    in rows 64:127, one 512-cycle pass for both) -> one [128, 2, 512] PSUM
    tile per key-tile; one [128, 1024] exp per key-tile covering both heads.
  - exp split across engines: 11/16 key-tiles on ScalarE (table exp writing
    fp16), 5/16 on the DVE as an integer Schraudolph exp (bits =
    round(SCH_A*score + SCH_B) written as uint16 = the fp16 bit pattern).
  - softmax row-sums come FREE from a 65th all-ones column appended to V
    (65-wide ctx matmul writes ctx rows 0:63 + the sum in row 64).
  - normalize: stage the sum row to partition 0 (reciprocal_approx_fast
    ignores the input partition offset), fast reciprocal, gpsimd partition
    broadcast, DVE multiplies (partition-shifted write for head B).
  - ctx PSUM is a 3-slot ring of SEPARATE tiles (tile-granular dependency
    tracking would otherwise serialize each qc behind the previous
    normalize); head B's accumulation lags 3 key-tiles so the slot handoff
    is off the critical path; scores+exp are software-pipelined one tile
    ahead across qc/pair boundaries.
  - the freed 8th PSUM bank hosts fillers that keep the PE busy (and the
    HAM clock-gate at 8/8): V projection streaming just ahead of pair 0,
    Q/K projections for pair p+1 during pair p, and the output projection
    for pair 3's previous q-chunk during pair 3.
  - PE warm-up: dummy matmuls during the initial x DMA flip the HAM
    clock-gate to 8/8 before the first projection.
"""

import numpy as np

B, N, C, H = 4, 2048, 1024, 16
D = C // H            # 64
G = 2                 # head-groups (tensor-parallel factor)
J = C // G            # 512 local channels
HL = H // G           # 8 local heads
CT = C // 128         # 8 c-tiles
JT = J // 128         # 4 local j-tiles (head pairs)
NT = N // 128         # 16 token tiles
KT = N // 128         # 16 key tiles
QC = 512              # q-chunk width
NQC = N // QC         # 4 q-chunks
HC = 512              # projection chunk width
NHC = N // HC         # 4 chunks
N_CORES = 8

# key-tiles whose exp runs on the DVE as an integer Schraudolph exp writing
# fp16 bit patterns (offloads ~1/3 of the exp work from the ScalarE).
# Pairs 0 and 3 carry extra DVE load (V-projection writes / output-projection
# copies), so they offload fewer tiles.
SCH_BY_PAIR = {0: (5, 11), 1: (2, 5, 8, 11, 14), 2: (2, 5, 8, 11, 14),
               3: (5, 11)}
SCH_A = float(1024 * 0.125 * np.log2(np.e))   # per raw-score unit
SCH_B = 15301.0                               # calibrated for fp16 bits

_CACHE = {}


def _build():
    import sys
    if "/opt/trn_rl_repo" not in sys.path:
        sys.path.insert(0, "/opt/trn_rl_repo")
    from contextlib import ExitStack
    import concourse.bacc as bacc
    import concourse.tile as tile
    from concourse import mybir

    f32 = mybir.dt.float32
    f16 = mybir.dt.float16
    u16 = mybir.dt.uint16
    Exp = mybir.ActivationFunctionType.Exp
    mult = mybir.AluOpType.mult
    add = mybir.AluOpType.add

    nc = bacc.Bacc("TRN2", target_bir_lowering=False, debug=False)

    xT_d = nc.dram_tensor("xT", [C, N], f16, kind="ExternalInput")
    wq_d = nc.dram_tensor("wq", [C, J], f16, kind="ExternalInput")
    wk_d = nc.dram_tensor("wk", [C, J], f16, kind="ExternalInput")
    wv_d = nc.dram_tensor("wv", [C, J], f16, kind="ExternalInput")
    wo_d = nc.dram_tensor("wo", [J, C], f16, kind="ExternalInput")
    bq_d = nc.dram_tensor("bq", [J], f32, kind="ExternalInput")
    bk_d = nc.dram_tensor("bk", [J], f32, kind="ExternalInput")
    bv_d = nc.dram_tensor("bv", [J], f32, kind="ExternalInput")
    y_d = nc.dram_tensor("y", [N, C], f32, kind="ExternalOutput")

    xT_r = xT_d.ap().rearrange("(ct p) n -> p ct n", p=128)

    with tile.TileContext(nc) as tc, ExitStack() as top:
        consts = top.enter_context(tc.tile_pool(name="consts", bufs=1))
        persist = top.enter_context(tc.tile_pool(name="persist", bufs=1))
        etp = top.enter_context(tc.tile_pool(name="etp", bufs=8))
        ysbp = top.enter_context(tc.tile_pool(name="ysb", bufs=3))
        rr1p = top.enter_context(tc.tile_pool(name="rr1p", bufs=3))
        rrbp = top.enter_context(tc.tile_pool(name="rrbp", bufs=3))

        # ---- persistent SBUF ----
        xt_t = persist.tile([128, CT, N], f16, tag="xt")
        qt_t = persist.tile([128, JT, N], f16, tag="qt")
        kt_t = persist.tile([128, JT, N], f16, tag="kt")
        v_t = persist.tile([128, KT, HL, 66], f16, tag="v")
        ctxT_t = persist.tile([128, JT, N], f16, tag="ctxT")
        wq_t = persist.tile([128, CT, J], f16, tag="wq")
        wk_t = persist.tile([128, CT, J], f16, tag="wk")
        wv_t = persist.tile([128, CT, J], f16, tag="wv")
        wo_t = persist.tile([128, JT, C], f16, tag="wo")

        bq_t = consts.tile([128, JT], f32, tag="bq")
        bk_t = consts.tile([128, JT], f32, tag="bk")
        bv_t = consts.tile([128, J], f32, tag="bv")
        warm_t = consts.tile([128, 8], f32, tag="warm")
        warm_o = consts.tile([128, 8], f16, tag="warmo")

        # ---- DMAs (x on the sync queue, weights on the gpsimd queue) ----
        for ct in range(CT):
            nc.sync.dma_start(out=xt_t[:, ct, :], in_=xT_r[:, ct, :])
        for ct in range(CT):
            nc.gpsimd.dma_start(out=wq_t[:, ct, :], in_=wq_d.ap()[ct * 128:(ct + 1) * 128, :])
            nc.gpsimd.dma_start(out=wk_t[:, ct, :], in_=wk_d.ap()[ct * 128:(ct + 1) * 128, :])
            nc.gpsimd.dma_start(out=wv_t[:, ct, :], in_=wv_d.ap()[ct * 128:(ct + 1) * 128, :])
        for jt in range(JT):
            nc.gpsimd.dma_start(out=wo_t[:, jt, :], in_=wo_d.ap()[jt * 128:(jt + 1) * 128, :])
        nc.sync.dma_start(out=bq_t[:], in_=bq_d.ap().rearrange("(t p) -> p t", p=128))
        nc.sync.dma_start(out=bk_t[:], in_=bk_d.ap().rearrange("(t p) -> p t", p=128))
        nc.sync.dma_start(
            out=bv_t[:], in_=bv_d.ap().unsqueeze(0).partition_broadcast(128).squeeze(1)
        )

        nc.vector.memset(warm_t[:], 0.0)
        nc.scalar.activation(warm_o[:], warm_t[:], Exp)  # load exp table early
        nc.vector.memset(v_t[:, :, :, 64:65], 1.0)       # ones col -> sum in row 64

        def qk_chunk_gen(jt, pool, h, on_scalar=False):
            ns = h * HC
            Ident = mybir.ActivationFunctionType.Identity
            for w_t, b_t, o_t in ((wq_t, bq_t, qt_t), (wk_t, bk_t, kt_t)):
                ps = pool.tile([128, HC], f32, tag="qk")
                for ct in range(CT):
                    nc.tensor.matmul(
                        ps[:], w_t[:, ct, jt * 128:(jt + 1) * 128],
                        xt_t[:, ct, ns:ns + HC],
                        start=(ct == 0), stop=(ct == CT - 1),
                    )
                    yield
                if on_scalar:
                    # keep the PSUM-release off the DVE queue (it would sit
                    # behind the normalize chain and stall the PE fillers)
                    nc.scalar.activation(
                        o_t[:, jt, ns:ns + HC], ps[:], Ident,
                        bias=b_t[:, jt:jt + 1],
                    )
                else:
                    nc.vector.tensor_scalar_add(
                        o_t[:, jt, ns:ns + HC], ps[:], b_t[:, jt:jt + 1]
                    )

        def yproj_gen(qc, pool):
            for nt in range(4 * qc, 4 * qc + 4):
                for cc in range(2):
                    y_ps = pool.tile([128, 512], f32, tag="qk")
                    for jt in range(JT):
                        nc.tensor.matmul(
                            y_ps[:],
                            ctxT_t[:, jt, nt * 128:(nt + 1) * 128],
                            wo_t[:, jt, cc * 512:(cc + 1) * 512],
                            start=(jt == 0), stop=(jt == JT - 1),
                        )
                        yield
                    y_sb = ysbp.tile([128, 512], f32, tag="ysb")
                    nc.vector.tensor_copy(y_sb[:], y_ps[:])
                    nc.sync.dma_start(
                        out=y_d.ap()[nt * 128:(nt + 1) * 128,
                                     cc * 512:(cc + 1) * 512],
                        in_=y_sb[:],
                    )

        def drain(gen, n=10 ** 9):
            for _ in range(n):
                if next(gen, "END") == "END":
                    return True
            return False

        def qk_pass(jt, pool):
            for h in range(NHC):
                drain(qk_chunk_gen(jt, pool, h))

        # ---- pass 0: Q/K for j-tile 0 (with PE warm-up during the x DMA:
        #      ~4us of dummy matmuls flips the HAM clock gate to 8/8) ----
        warm16 = consts.tile([128, 512], f16, tag="warm16")
        nc.vector.memset(warm16[:], 0.0)
        with tc.tile_pool(name="p0ps", bufs=2, space="PSUM") as p0ps:
            wu_ps = p0ps.tile([128, 512], f32, tag="qk")
            for r in range(24):
                nc.tensor.matmul(
                    wu_ps[:], warm16[:, 0:128], warm16[:],
                    start=(r == 0), stop=(r == 23),
                )
            qk_pass(0, p0ps)
            for nt in range(4):
                v_ps = p0ps.tile([128, J], f32, tag="qk")
                for ct in range(CT):
                    nc.tensor.matmul(
                        v_ps[:], xt_t[:, ct, nt * 128:(nt + 1) * 128],
                        wv_t[:, ct, :], start=(ct == 0), stop=(ct == CT - 1),
                    )
                nc.vector.tensor_tensor(
                    v_t[:, nt, :, 0:64],
                    v_ps[:].rearrange("p (h d) -> p h d", h=HL),
                    bv_t[:].rearrange("p (h d) -> p h d", h=HL),
                    add,
                )

        # ---- attention (pair p) interleaved with Q/K projections (p+1);
        #      the V projection streams through the qki bank just ahead of
        #      its first consumers in (pair 0, qc 0) ----
        with (
            tc.tile_pool(name="stp", bufs=2, space="PSUM") as stp,
            tc.tile_pool(name="cxps", bufs=1, space="PSUM") as cxps,
            tc.tile_pool(name="qki", bufs=1, space="PSUM") as qki,
        ):
            for nt in range(4, NT):
                v_ps = qki.tile([128, J], f32, tag="qk")
                for ct in range(CT):
                    nc.tensor.matmul(
                        v_ps[:], xt_t[:, ct, nt * 128:(nt + 1) * 128],
                        wv_t[:, ct, :], start=(ct == 0), stop=(ct == CT - 1),
                    )
                nc.vector.tensor_tensor(
                    v_t[:, nt, :, 0:64],
                    v_ps[:].rearrange("p (h d) -> p h d", h=HL),
                    bv_t[:].rearrange("p (h d) -> p h d", h=HL),
                    add,
                )
            # 3-slot ring as separate tiles (separate dependency domains —
            # a single tile would serialize each qc's ctx behind the
            # previous qc's normalize reads)
            cx_slots = []
            for s in range(3):
                cx_slot = cxps.tile([128, QC], f32, tag=f"cx{s}", name=f"cx{s}")
                cx_slots.append(cx_slot)

            def emit_scores_exp(p, qc, k):
                qs = qc * QC
                st_ps = stp.tile([128, 2, QC], f32, tag="st")
                nc.tensor.matmul(
                    st_ps[:, 0, :],
                    kt_t[0:64, p, k * 128:(k + 1) * 128],
                    qt_t[0:64, p, qs:qs + QC],
                    start=True, stop=True,
                )
                nc.tensor.matmul(
                    st_ps[:, 1, :],
                    kt_t[64:128, p, k * 128:(k + 1) * 128],
                    qt_t[64:128, p, qs:qs + QC],
                    start=True, stop=True,
                )
                et_t = etp.tile([128, 2, QC], f16, tag="et")
                if k in SCH_BY_PAIR[p]:
                    nc.vector.tensor_scalar(
                        et_t[:].bitcast(u16), st_ps[:], SCH_A, SCH_B, mult, add
                    )
                else:
                    nc.scalar.activation(et_t[:], st_ps[:], Exp, scale=0.125)
                return et_t

            units = [(p, qc) for p in range(JT) for qc in range(NQC)]
            et_next = emit_scores_exp(0, 0, 0)
            for ui, (p, qc) in enumerate(units):
                hA, hB = 2 * p, 2 * p + 1
                i = ui
                sA, sB = (2 * i) % 3, (2 * i + 1) % 3
                qs = qc * QC
                if p + 1 < JT:
                    filler = qk_chunk_gen(p + 1, qki, qc)
                    per_k = 3
                elif qc > 0:
                    filler = yproj_gen(qc - 1, qki)
                    per_k = 3
                else:
                    filler = iter(())
                    per_k = 0
                pend_b = []
                for k in range(KT):
                    et_t = et_next
                    # software pipeline: launch the NEXT tile's scores+exp
                    # (crossing qc/pair boundaries) before this tile's ctx
                    if k + 1 < KT:
                        et_next = emit_scores_exp(p, qc, k + 1)
                    elif ui + 1 < len(units):
                        p2, qc2 = units[ui + 1]
                        et_next = emit_scores_exp(p2, qc2, 0)
                    first, last = (k == 0), (k == KT - 1)
                    with tc.high_priority():
                        nc.tensor.matmul(
                            cx_slots[sA][0:65, :], v_t[:, k, hA, 0:65],
                            et_t[:, 0, :], start=first, stop=last,
                        )
                    # lag head B by 3 k-tiles: its k=0 matmul reuses the
                    # ring slot the previous qc's normalize just released
                    pend_b.append((k, et_t))
                    if len(pend_b) > 3:
                        kb, et_b = pend_b.pop(0)
                        with tc.high_priority():
                            nc.tensor.matmul(
                                cx_slots[sB][0:65, :], v_t[:, kb, hB, 0:65],
                                et_b[:, 1, :], start=(kb == 0),
                                stop=(kb == KT - 1),
                            )
                    drain(filler, per_k)
                for kb, et_b in pend_b:
                    with tc.high_priority():
                        nc.tensor.matmul(
                            cx_slots[sB][0:65, :], v_t[:, kb, hB, 0:65],
                            et_b[:, 1, :], start=(kb == 0),
                            stop=(kb == KT - 1),
                        )
                drain(filler)
                # normalize: slot A first (it is reused soonest).
                # reciprocal_approx_fast only reads partition 0 correctly,
                # so stage the sum row there first.
                for s, h0, pr in ((sA, 0, hA), (sB, 1, hB)):
                    rr1_t = rr1p.tile([1, QC], f32, tag="rr1")
                    sc_t = rr1p.tile([1, QC], f32, tag="sc")
                    nc.vector.tensor_copy(sc_t[0:1, :], cx_slots[s][64:65, :])
                    nc.vector.reciprocal_approx_fast(
                        rr1_t[0:1, :], sc_t[0:1, :]
                    )
                    rrb_t = rrbp.tile([128, QC], f32, tag="rrb")
                    nc.gpsimd.partition_broadcast(rrb_t[:], rr1_t[0:1, :])
                    po = 64 * h0
                    nc.vector.tensor_tensor(
                        ctxT_t[po:po + 64, p, qs:qs + QC],
                        cx_slots[s][0:64, :],
                        rrb_t[po:po + 64, :], mult,
                    )
        with tc.tile_pool(name="yfl", bufs=4, space="PSUM") as yfl:
            # dependency-free bridge matmuls: the flush must wait ~4us for the
            # last normalize, which is one full HAM window — without these the
            # clock gate re-throttles and the flush runs at 1.2 GHz
            br_ps = yfl.tile([128, 512], f32, tag="qk")
            for r in range(14):
                nc.tensor.matmul(
                    br_ps[:], warm16[:, 0:128], warm16[:],
                    start=(r == 0), stop=(r == 13),
                )
            with tc.high_priority():
                drain(yproj_gen(NQC - 1, yfl))

    nc.compile()
    return nc


def _get_module():
    if "nc" not in _CACHE:
        _CACHE["nc"] = _build()
    return _CACHE["nc"]


def _in_maps(x, Wq, bq, Wk, bk, Wv, bv, Wo):
    f16 = np.float16
    maps = []
    for c in range(N_CORES):
        b, g = divmod(c, 2)
        js = slice(g * J, (g + 1) * J)
        maps.append({
            "xT": np.ascontiguousarray(x[b].T.astype(f16)),
            "wq": np.ascontiguousarray(Wq[:, js].astype(f16)),
            "wk": np.ascontiguousarray(Wk[:, js].astype(f16)),
            "wv": np.ascontiguousarray(Wv[:, js].astype(f16)),
            "wo": np.ascontiguousarray(Wo[js, :].astype(f16)),
            "bq": np.ascontiguousarray(bq[js].astype(np.float32)),
            "bk": np.ascontiguousarray(bk[js].astype(np.float32)),
            "bv": np.ascontiguousarray(bv[js].astype(np.float32)),
        })
    return maps


def kernel(x, Wq, bq, Wk, bk, Wv, bv, Wo, bo, **_unused):
    import sys
    if "/opt/trn_rl_repo" not in sys.path:
        sys.path.insert(0, "/opt/trn_rl_repo")
    from concourse.bass_utils import run_bass_kernel_spmd

    x = np.asarray(x, dtype=np.float32)
    Wq = np.asarray(Wq, dtype=np.float32)
    Wk = np.asarray(Wk, dtype=np.float32)
    Wv = np.asarray(Wv, dtype=np.float32)
    Wo = np.asarray(Wo, dtype=np.float32)
    bq = np.asarray(bq, dtype=np.float32)
    bk = np.asarray(bk, dtype=np.float32)
    bv = np.asarray(bv, dtype=np.float32)
    bo = np.asarray(bo, dtype=np.float32)

    nc = _get_module()
    res = run_bass_kernel_spmd(
        nc, _in_maps(x, Wq, bq, Wk, bk, Wv, bv, Wo), list(range(N_CORES))
    )
    out = np.empty((B, N, C), dtype=np.float32)
    for b in range(B):
        out[b] = res.results[2 * b]["y"] + res.results[2 * b + 1]["y"] + bo
    return out
